# revision 1
# baseline (speedup 1.0000x reference)
"""Swin-style windowed-attention block on 8 TRN2 NeuronCores (data-parallel over batch).

Self-contained: host-side prep (fold norms/scale into weights, bias+mask tables,
bf16 casts, transposes, zero-pad windows 49->64 tokens) + a fused Bass/Tile kernel.

Per-core layout: tokens are processed in window PAIRS; each window is padded to
64 tokens so a pair fills 128 partitions exactly (win0 @ rows 0:49, win1 @ rows
64:113, pad rows zero/garbage and never stored). All matmul operands then sit at
legal PE array-tile bases (0/32/64/96).
"""

import sys

sys.path.insert(0, "/opt/trn_rl_repo")

import numpy as np
import ml_dtypes

import concourse.bass as bass
import concourse.bacc as bacc
import concourse.tile as tile
import concourse.mybir as mybir
from concourse.bass_utils import run_bass_kernel_spmd

BF16 = ml_dtypes.bfloat16
FP8 = ml_dtypes.float8_e4m3
FP32 = mybir.dt.float32
BF16_DT = mybir.dt.bfloat16
FP8_DT = mybir.dt.float8e4
W8SCALE = 64.0

# ---- static geometry ----
WH, WW = 7, 7
S = 49                     # valid tokens per window
SP = 64                    # padded tokens per window
C = 256                    # channels
NH = 8                     # heads
HD = 32                    # head dim
NWIN = 256                 # windows per batch image
B = 8                      # batch == number of cores
GRID = 16                  # 16x16 window grid
SCALE = HD ** -0.5
EPS = 1e-5
MASK_VAL = -30000.0

NPAIR = NWIN // 2          # 128 window pairs per core
PPC = 8                    # pairs per chunk
NCHUNK = NPAIR // PPC      # 16 chunks
TPP = 2 * SP               # 128 padded tokens per pair
VPP = 2 * S                # 98 valid tokens per pair
TPC = PPC * TPP            # 1024 padded tokens per chunk
VPC = PPC * VPP            # 784 valid tokens per chunk
NTOK = NWIN * S            # 12544 valid tokens per core
NTOKP = NPAIR * TPP        # 16384 padded tokens per core
SLAB = 4 * TPP             # 512-wide moving slab (4 pairs)
SLABS = TPC // SLAB        # 2 slabs per chunk

ActF = mybir.ActivationFunctionType
Alu = mybir.AluOpType


# --------------------------------------------------------------------------
# host-side preparation
# --------------------------------------------------------------------------

def _relative_position_index():
    ch, cw = np.arange(WH), np.arange(WW)
    coords = np.stack(np.meshgrid(ch, cw, indexing="ij")).reshape(2, -1)
    rel = coords[:, :, None] - coords[:, None, :]
    rel = rel.transpose(1, 2, 0).astype(np.int64)
    rel[..., 0] += WH - 1
    rel[..., 1] += WW - 1
    rel[..., 0] *= 2 * WW - 1
    return rel.sum(-1)                                    # (S, S)


def _window_mask_types():
    """Per-window mask type: 0 none, 1 bottom-row, 2 right-col, 3 corner."""
    h = w = GRID
    s1, s2 = WH - WH // 2, WW - WW // 2
    m = np.zeros((h, w, WH, WW, WH, WW), dtype=bool)
    m[-1, :, :s1, :, s1:, :] = True
    m[-1, :, s1:, :, :s1, :] = True
    m[:, -1, :, :s2, :, s2:] = True
    m[:, -1, :, s2:, :, :s2] = True
    m = m.reshape(h * w, S, S)
    types = np.zeros(NWIN, dtype=np.int64)
    rr, cc = np.divmod(np.arange(NWIN), GRID)
    types[(rr == GRID - 1) & (cc < GRID - 1)] = 1
    types[(rr < GRID - 1) & (cc == GRID - 1)] = 2
    types[(rr == GRID - 1) & (cc == GRID - 1)] = 3
    masks = np.zeros((4, S, S), dtype=np.float32)
    masks[1] = np.where(m[GRID * (GRID - 1)], MASK_VAL, 0.0)
    masks[2] = np.where(m[GRID - 1], MASK_VAL, 0.0)
    masks[3] = np.where(m[NWIN - 1], MASK_VAL, 0.0)
    return types, masks


def _pair_types():
    types, _ = _window_mask_types()
    combos = []
    ptype = np.zeros(NPAIR, dtype=np.int64)
    for j in range(NPAIR):
        c = (int(types[2 * j]), int(types[2 * j + 1]))
        if c not in combos:
            combos.append(c)
        ptype[j] = combos.index(c)
    assert len(combos) <= 4, combos
    while len(combos) < 4:
        combos.append((0, 0))
    return ptype, combos


_PTYPE, _PCOMBOS = _pair_types()


def _tile_kxoc(wT):
    """[K, OC] -> [128, K//128, OC] with K = 128*kt + p."""
    K, OC = wT.shape
    return np.ascontiguousarray(wT.reshape(K // 128, 128, OC).transpose(1, 0, 2))


def host_prep(inputs):
    x = np.asarray(inputs["x"], dtype=np.float32)          # (B, N, S, C)
    qkv_w = np.asarray(inputs["qkv_w"], dtype=np.float32)
    qkv_b = np.asarray(inputs["qkv_b"], dtype=np.float32)
    proj_w = np.asarray(inputs["proj_w"], dtype=np.float32)
    proj_b = np.asarray(inputs["proj_b"], dtype=np.float32)
    n1g = np.asarray(inputs["norm1_g"], dtype=np.float32)
    n1b = np.asarray(inputs["norm1_b"], dtype=np.float32)
    n2g = np.asarray(inputs["norm2_g"], dtype=np.float32)
    n2b = np.asarray(inputs["norm2_b"], dtype=np.float32)
    w1 = np.asarray(inputs["mlp_w1"], dtype=np.float32)
    b1 = np.asarray(inputs["mlp_b1"], dtype=np.float32)
    w2 = np.asarray(inputs["mlp_w2"], dtype=np.float32)
    b2 = np.asarray(inputs["mlp_b2"], dtype=np.float32)
    table = np.asarray(inputs["bias_table"], dtype=np.float32)

    # fold layernorm affine into the following matmuls
    qkv_w_f = qkv_w * n1g[None, :]
    qkv_b_f = qkv_b + qkv_w @ n1b
    w1_f = w1 * n2g[None, :]
    b1_f = b1 + w1 @ n2b

    wq = qkv_w_f[0:C] * SCALE
    bq = qkv_b_f[0:C] * SCALE
    wk = qkv_w_f[C:2 * C]
    bk = qkv_b_f[C:2 * C]
    wv = qkv_w_f[2 * C:3 * C]
    bv = qkv_b_f[2 * C:3 * C]

    common = {
        "wq": _tile_kxoc(wq.T).astype(BF16),
        "wk": _tile_kxoc(wk.T).astype(BF16),
        "wv": _tile_kxoc(wv.T).astype(BF16),
        "wp": _tile_kxoc(proj_w.T).astype(BF16),
        "w1": _tile_kxoc(w1_f.T * W8SCALE).astype(FP8),
        "w2": _tile_kxoc(w2.T * W8SCALE).astype(FP8),
        "bq": np.ascontiguousarray(bq.reshape(2, 128).T).astype(np.float32),
        "bk": np.ascontiguousarray(bk.reshape(2, 128).T).astype(np.float32),
        "b1": np.ascontiguousarray(b1_f.reshape(8, 128).T).astype(np.float32),
    }

    # augmented-K bias rows: k'[32+i, h, (r), t] = (bias_h + mask)[i, t]
    # (i indexes s; q' carries identity rows so the matmul adds bias[s, t]).
    # t-pad columns get MASK_VAL so exp() zeroes padded key rows.
    rel = _relative_position_index()
    bias_sht = table[rel].transpose(2, 0, 1)               # [h, s, t]
    _, masks = _window_mask_types()                        # [4, s, t]
    bmps = np.full((S, 4, NH, 2, SP), MASK_VAL, dtype=np.float32)
    for pt, (t0, t1) in enumerate(_PCOMBOS):
        for h in range(NH):
            bmps[:, pt, h, 0, 0:S] = bias_sht[h] + masks[t0]
            bmps[:, pt, h, 1, 0:S] = bias_sht[h] + masks[t1]
    common["bmps"] = bmps.astype(BF16)
    qid = np.zeros((S, NH, 2, SP), dtype=np.float32)
    for h in range(NH):
        for r in range(2):
            qid[:, h, r, 0:S] = np.eye(S, dtype=np.float32)
    common["qid"] = qid.astype(BF16)

    extra = {
        "bv_nz": bool(np.any(bv != 0.0)),
        "pb_nz": bool(np.any(proj_b != 0.0)),
        "b2_nz": bool(np.any(b2 != 0.0)),
    }
    if extra["bv_nz"]:
        common["bvbc"] = np.tile(bv[None, :], (128, 1)).astype(np.float32)
    if extra["pb_nz"]:
        common["pbbc"] = np.tile(proj_b[None, :], (128, 1)).astype(np.float32)
    if extra["b2_nz"]:
        common["b2bc"] = np.tile(b2[None, :], (128, 1)).astype(np.float32)

    in_maps = []
    for b in range(B):
        m = dict(common)
        xp = np.zeros((NWIN, SP, C), dtype=np.float32)
        xp[:, :S, :] = x[b]
        m["x"] = xp.reshape(NTOKP, C)
        in_maps.append(m)
    return in_maps, extra


# --------------------------------------------------------------------------
# kernel builder
# --------------------------------------------------------------------------

def build_program(n_pairs=NPAIR, bv_nz=False, pb_nz=False, b2_nz=False,
                  use_dma_transpose=True):
    assert n_pairs % PPC == 0
    n_chunks = n_pairs // PPC

    nc = bacc.Bacc("TRN2", target_bir_lowering=False, debug=False)

    ext = {}
    ext["x"] = nc.dram_tensor("x", [n_pairs * TPP, C], FP32, kind="ExternalInput")
    ext["out"] = nc.dram_tensor("out", [n_pairs * VPP, C], FP32, kind="ExternalOutput")
    ext["wq"] = nc.dram_tensor("wq", [128, 2, C], BF16_DT, kind="ExternalInput")
    ext["wk"] = nc.dram_tensor("wk", [128, 2, C], BF16_DT, kind="ExternalInput")
    ext["wv"] = nc.dram_tensor("wv", [128, 2, C], BF16_DT, kind="ExternalInput")
    ext["wp"] = nc.dram_tensor("wp", [128, 2, C], BF16_DT, kind="ExternalInput")
    ext["w1"] = nc.dram_tensor("w1", [128, 2, 4 * C], FP8_DT, kind="ExternalInput")
    ext["w2"] = nc.dram_tensor("w2", [128, 8, C], FP8_DT, kind="ExternalInput")
    ext["bq"] = nc.dram_tensor("bq", [128, 2], FP32, kind="ExternalInput")
    ext["bk"] = nc.dram_tensor("bk", [128, 2], FP32, kind="ExternalInput")
    ext["b1"] = nc.dram_tensor("b1", [128, 8], FP32, kind="ExternalInput")
    ext["bmps"] = nc.dram_tensor("bmps", [S, 4, NH, 2, SP], BF16_DT, kind="ExternalInput")
    ext["qid"] = nc.dram_tensor("qid", [S, NH, 2, SP], BF16_DT, kind="ExternalInput")
    ext["bvbc"] = nc.dram_tensor("bvbc", [128, C], FP32, kind="ExternalInput") if bv_nz else None
    ext["pbbc"] = nc.dram_tensor("pbbc", [128, C], FP32, kind="ExternalInput") if pb_nz else None
    ext["b2bc"] = nc.dram_tensor("b2bc", [128, C], FP32, kind="ExternalInput") if b2_nz else None

    with tile.TileContext(nc) as tc:
        _body(tc, n_chunks, ext, use_dma_transpose)

    nc.compile()
    return nc


def _body(tc, n_chunks, ext, use_dma_transpose):
    nc = tc.nc
    import contextlib
    with contextlib.ExitStack() as ctx:
        const = ctx.enter_context(tc.tile_pool(name="const", bufs=1))
        cst = {}
        for name, shape, dt in (
            ("wq", [128, 2, C], BF16_DT), ("wk", [128, 2, C], BF16_DT),
            ("wv", [128, 2, C], BF16_DT), ("wp", [128, 2, C], BF16_DT),
            ("w1", [128, 2, 4 * C], FP8_DT), ("w2", [128, 8, C], FP8_DT),
            ("bq", [128, 2], FP32), ("bk", [128, 2], FP32),
            ("b1", [128, 8], FP32),
            ("bvbc", [128, C], FP32), ("pbbc", [128, C], FP32),
            ("b2bc", [128, C], FP32),
        ):
            if ext.get(name) is None:
                cst[name] = None
                continue
            t = const.tile(shape, dt, tag=name)
            nc.sync.dma_start(out=t[:], in_=ext[name].ap())
            cst[name] = t
        ident = const.tile([128, 128], BF16_DT, tag="ident")
        from concourse.masks import make_identity
        make_identity(nc, ident[:])
        cst["ident"] = ident
        eps_sb = const.tile([128, 1], FP32, tag="eps")
        nc.vector.memset(eps_sb[:], EPS)
        cst["eps"] = eps_sb

        pools = {}
        for name, bufs in (("xp", 3), ("xnp", 2), ("tp", 2), ("qkp", 2),
                           ("vsp", 3), ("vp", 2), ("etp", 4),
                           ("atp", 1), ("x2p", 2), ("hp", 1), ("statp", 2)):
            pools[name] = ctx.enter_context(tc.tile_pool(name=name, bufs=bufs))

        # q'/k' augmented tiles (rows 0:32 data, 32:81 identity/bias), one
        # pair per chunk-parity for double buffering.
        qk_aug = []
        for par in range(2):
            qa = const.tile([32 + S, NH, PPC, TPP], BF16_DT, tag=f"qaug{par}")
            ka = const.tile([32 + S, NH, PPC, TPP], BF16_DT, tag=f"kaug{par}")
            for h in range(NH):
                nc.sync.dma_start(out=qa[32:32 + S, h, :, :],
                                  in_=_bc_pairs(ext["qid"].ap()[:, h, :, :]))
            qk_aug.append((qa, ka))
        kpat_state = [None, None]
        for name, bufs in (("ps_wide", 2), ("ps_tok", 2),
                           ("ps_S", 2), ("ps_A", 2)):
            pools[name] = ctx.enter_context(
                tc.tile_pool(name=name, bufs=bufs, space="PSUM"))

        for ci in range(n_chunks):
            _chunk(tc, ci, ext, cst, pools, use_dma_transpose,
                   qk_aug, kpat_state)


def dst_aug_slice(aug, h, jsl):
    return aug[0:32, h, jsl, :]


def _bc_pairs(ap3):
    """[S, 2, SP] -> [S, PPC, 2, SP] with stride-0 pair axis (for DMA)."""
    return bass.AP(tensor=ap3.tensor, offset=ap3.offset,
                   ap=[ap3.ap[0], [0, PPC]] + list(ap3.ap[1:]))


def _transpose_pair_to(nc, src_ap_fn, dst_tile, j, ident, ps_tr, use_dma_t,
                       drain_dve=False):
    """Transpose a [128, 256] bf16 token-major pair-tile into dst[:, :, j, :]."""
    if use_dma_t:
        for ch in range(2):
            nc.sync.dma_start_transpose(out=dst_tile[:, ch, j, :], in_=src_ap_fn(ch))
    else:
        ps = ps_tr.tile([128, 2, 128], BF16_DT, tag="tok")
        for ch in range(2):
            nc.tensor.transpose(ps[:, ch, :], src_ap_fn(ch), ident[:, :])
        if drain_dve:
            nc.vector.tensor_copy(out=dst_tile[:, :, j, :], in_=ps[:, :, :])
        else:
            nc.scalar.activation(dst_tile[:, :, j, :], ps[:, :, :], ActF.Copy)


def _layernorm(nc, pools, x_t, xn_t, eps_sb):
    """x_t [128, PPC, 256] f32 -> xn_t bf16 ((x-mu)*rstd), stats on DVE."""
    statp = pools["statp"]
    mv = statp.tile([128, PPC, 2], FP32, tag="mv")
    for j in range(PPC):
        st = statp.tile([128, 6], FP32, tag="bnst")
        nc.vector.bn_stats(st[:], x_t[:, j, :])
        nc.vector.bn_aggr(mv[:, j, :], st[:])
    rstd = statp.tile([128, PPC], FP32, tag="rstd")
    nc.scalar.activation(rstd[:], mv[:, :, 1], ActF.Sqrt, bias=eps_sb[:])
    nc.vector.reciprocal(rstd[:], rstd[:])
    for j in range(PPC):
        nc.vector.tensor_scalar(
            out=xn_t[:, j, :], in0=x_t[:, j, :],
            scalar1=mv[:, j, 0:1], scalar2=rstd[:, j:j + 1],
            op0=Alu.subtract, op1=Alu.mult,
        )


def _chunk(tc, ci, ext, cst, pools, use_dma_t, qk_aug, kpat_state):
    nc = tc.nc
    ident = cst["ident"]
    ps_tr = pools["ps_tok"]

    # ---- phase A: load + LN1 + PE transpose ----
    x_t = pools["xp"].tile([128, PPC, C], FP32, tag="xo")
    nc.sync.dma_start(
        out=x_t[:],
        in_=ext["x"][ci * TPC:(ci + 1) * TPC, :].rearrange("(j p) c -> p j c", p=TPP),
    )
    xn_t = pools["xnp"].tile([128, PPC, C], BF16_DT, tag="xn")
    _layernorm(nc, pools, x_t, xn_t, cst["eps"])
    xnT = pools["tp"].tile([128, 2, PPC, TPP], BF16_DT, tag="xnT")
    for j in range(PPC):
        _transpose_pair_to(nc, lambda ch: xn_t[:, j, 128 * ch:128 * (ch + 1)],
                           xnT, j, ident, ps_tr, False, drain_dve=(j % 2 == 0))

    # ---- phase B: QKV ----
    qT = pools["qkp"].tile([128, 2, PPC, TPP], BF16_DT, tag="qT")
    kT = pools["qkp"].tile([128, 2, PPC, TPP], BF16_DT, tag="kT")
    qa, ka = qk_aug[ci % 2]
    pattern = tuple(int(_PTYPE[ci * PPC + j]) for j in range(PPC))
    if kpat_state[ci % 2] != pattern:
        for j in range(PPC):
            nc.sync.dma_start(
                out=ka[32:32 + S, :, j, :],
                in_=ext["bmps"].ap()[:, pattern[j], :, :, :])
        kpat_state[ci % 2] = pattern
    for s2 in range(SLABS):
        jsl = slice(4 * s2, 4 * s2 + 4)
        for (dst, wname, bname), aug in (((qT, "wq", "bq"), qa),
                                         ((kT, "wk", "bk"), ka)):
            w_sb, b_sb = cst[wname], cst[bname]
            for octl in range(2):
                ps = pools["ps_wide"].tile([128, SLAB], FP32, tag="wide")
                for kt in range(2):
                    nc.tensor.matmul(
                        ps[:],
                        lhsT=w_sb[:, kt, 128 * octl:128 * (octl + 1)],
                        rhs=xnT[:, kt, jsl, :],
                        start=(kt == 0), stop=(kt == 1),
                    )
                nc.scalar.activation(
                    dst[:, octl, jsl, :], ps[:],
                    ActF.Identity, bias=b_sb[:, octl:octl + 1],
                )
    for ti, (dst, srct) in enumerate(((qa, qT), (ka, kT))):
        for h in range(NH):
            hh = 32 * (h % 4)
            eng = nc.scalar if ti == 0 else nc.sync
            eng.dma_start(out=dst[0:32, h, :, :],
                          in_=srct[hh:hh + 32, h // 4, :, :])

    # v: token-major psum -> bf16 stage -> both windows shifted to base 0
    vp = pools["vp"].tile([SP, 2, PPC, NH, HD + 1], BF16_DT, tag="vp")
    nc.vector.memset(vp[:, :, :, :, 0:1], 0.0)
    nc.vector.memset(vp[0:S, :, :, :, 0:1], 1.0)
    for j in range(PPC):
        ps = pools["ps_tok"].tile([128, C], FP32, tag="tok")
        for kt in range(2):
            nc.tensor.matmul(
                ps[:], lhsT=xnT[:, kt, j, :], rhs=cst["wv"][:, kt, :],
                start=(kt == 0), stop=(kt == 1),
            )
        vst = pools["vsp"].tile([128, C], BF16_DT, tag="vst")
        nc.scalar.activation(vst[:], ps[:], ActF.Copy)
        if cst["bvbc"] is not None:
            for r in range(2):
                sl = slice(SP * r, SP * r + S)
                nc.vector.tensor_add(vst[sl], vst[sl], cst["bvbc"][sl])
        for r in range(2):
            nc.sync.dma_start(
                out=vp[:, r, j, :, 1:HD + 1],
                in_=vst[SP * r:SP * (r + 1), :].rearrange(
                    "p (h d) -> p h d", h=NH),
            )

    # ---- phase C: attention ----
    attn_t = pools["atp"].tile([128, PPC, C], BF16_DT, tag="attn")
    for j in range(PPC):
        psa = pools["ps_A"].tile([128, NH, HD + 1], FP32, tag="A")
        for r in range(2):
            pss = pools["ps_S"].tile([SP, NH, SP], FP32, tag="S")
            for h in range(NH):
                nc.tensor.matmul(
                    pss[:, h, :],
                    lhsT=ka[:, h, j, SP * r:SP * (r + 1)],
                    rhs=qa[:, h, j, SP * r:SP * (r + 1)],
                    start=True, stop=True,
                )
            et = pools["etp"].tile([SP, NH, SP], BF16_DT, tag="et")
            nc.scalar.activation(et[:], pss[:], ActF.Exp)
            for h in range(NH):
                nc.tensor.matmul(
                    psa[SP * r:SP * (r + 1), h, :],
                    lhsT=et[:, h, :],
                    rhs=vp[:, r, j, h, :],
                    start=True, stop=True,
                )
        rec = pools["statp"].tile([128, NH], FP32, tag="rec")
        nc.vector.tensor_scalar_max(out=rec[:], in0=psa[:, :, 0], scalar1=1e-30)
        nc.vector.reciprocal(rec[:], rec[:])
        rec_s = rec[:]
        rec_b = bass.AP(tensor=rec_s.tensor, offset=rec_s.offset,
                        ap=list(rec_s.ap) + [[0, HD]])
        nc.vector.tensor_mul(
            attn_t[:, j, :].rearrange("p (h d) -> p h d", h=NH),
            psa[:, :, 1:HD + 1], rec_b,
        )

    # ---- phase D: transpose attn ----
    attnT = pools["tp"].tile([128, 2, PPC, TPP], BF16_DT, tag="attnT")
    for j in range(PPC):
        _transpose_pair_to(nc, lambda ch: attn_t[:, j, 128 * ch:128 * (ch + 1)],
                           attnT, j, ident, ps_tr, False, drain_dve=(j % 2 == 0))

    # ---- phase E: proj + resid1 + LN2 + transpose ----
    x2_t = pools["x2p"].tile([128, PPC, C], FP32, tag="x2")
    for j in range(PPC):
        ps = pools["ps_tok"].tile([128, C], FP32, tag="tok")
        for kt in range(2):
            nc.tensor.matmul(
                ps[:], lhsT=attnT[:, kt, j, :], rhs=cst["wp"][:, kt, :],
                start=(kt == 0), stop=(kt == 1),
            )
        if cst["pbbc"] is not None:
            nc.vector.tensor_add(ps[:], ps[:], cst["pbbc"][:])
        nc.vector.tensor_add(x2_t[:, j, :], ps[:], x_t[:, j, :])

    xn2_t = pools["xnp"].tile([128, PPC, C], BF16_DT, tag="xn")
    _layernorm(nc, pools, x2_t, xn2_t, cst["eps"])
    xn2T = pools["tp"].tile([128, 2, PPC, TPP], FP8_DT, tag="xn2T")
    for j in range(PPC):
        _transpose_pair_to(nc, lambda ch: xn2_t[:, j, 128 * ch:128 * (ch + 1)],
                           xn2T, j, ident, ps_tr, False, drain_dve=(j % 2 == 0))

    # ---- phase F: MLP ----
    hT = pools["hp"].tile([128, 8, PPC, TPP], FP8_DT, tag="hT")
    for s2 in range(SLABS):
        jsl = slice(4 * s2, 4 * s2 + 4)
        for m in range(8):
            ps = pools["ps_wide"].tile([128, SLAB], FP32, tag="wide")
            nc.tensor.matmul(
                ps[:], lhsT=cst["w1"][:, :, 128 * m:128 * (m + 1)],
                rhs=xn2T[:, :, jsl, :],
                start=True, stop=True,
                perf_mode=mybir.MatmulPerfMode.DoubleRow,
            )
            nc.scalar.activation(
                hT[:, m, jsl, :], ps[:], ActF.Gelu, bias=cst["b1"][:, m:m + 1],
                scale=1.0 / W8SCALE,
            )

    out_t = pools["xp"].tile([128, PPC, C], FP32, tag="xo")
    for j in range(PPC):
        ps = pools["ps_tok"].tile([128, C], FP32, tag="tok")
        for k2 in range(4):
            nc.tensor.matmul(
                ps[:], lhsT=hT[:, 2 * k2:2 * k2 + 2, j, :],
                rhs=cst["w2"][:, 2 * k2:2 * k2 + 2, :],
                start=(k2 == 0), stop=(k2 == 3),
                perf_mode=mybir.MatmulPerfMode.DoubleRow,
            )
        if cst["b2bc"] is not None:
            nc.vector.tensor_add(ps[:], ps[:], cst["b2bc"][:])
        nc.vector.scalar_tensor_tensor(
            out=out_t[:, j, :], in0=ps[:], scalar=1.0 / W8SCALE,
            in1=x2_t[:, j, :], op0=Alu.mult, op1=Alu.add,
        )

    # compact output: window (2j+r) valid rows SP*r : SP*r+S
    for r in range(2):
        dst = ext["out"][ci * VPC + r * S:, :]
        dst_ap = bass.AP(
            tensor=dst.tensor, offset=dst.offset,
            ap=[[C, S], [2 * S * C, PPC], [1, C]],
        )
        nc.sync.dma_start(out=dst_ap, in_=out_t[SP * r:SP * r + S, :, :])


# --------------------------------------------------------------------------
# entry point
# --------------------------------------------------------------------------

_CACHE = {}


def _get_program(key_flags):
    if key_flags not in _CACHE:
        _CACHE[key_flags] = build_program(
            NPAIR, bv_nz=key_flags[0], pb_nz=key_flags[1], b2_nz=key_flags[2],
            use_dma_transpose=key_flags[3],
        )
    return _CACHE[key_flags]


def kernel(**inputs):
    in_maps, extra = host_prep(inputs)
    nc = _get_program((extra["bv_nz"], extra["pb_nz"], extra["b2_nz"], True))
    res = run_bass_kernel_spmd(nc, in_maps, core_ids=list(range(B)))
    out = np.stack([res.results[i]["out"] for i in range(B)], axis=0)
    return out.reshape(B, NWIN, S, C).astype(np.float32)

